# revision 37
# baseline (speedup 1.0000x reference)
"""BERT self-attention (B=2, S=2048, H=1024, 16 heads) on 8 TRN2 NeuronCores.

Sharding: tensor-parallel over heads - 2 heads per core. Each core computes
Q/K/V projections for its head slice (contraction over the full hidden dim),
then attention for its (batch, head) pairs, producing the context transposed
[2*64, B*S]. The host concatenates the 8 per-core slices into [B, S, H].

Device-side layout choices:
  - X is fed pre-transposed and pre-blocked ([128, tb, ci, 512]) so each
    512-token block is one contiguous 8KB-per-partition DMA descriptor.
  - Scores are computed transposed (S^T = K Q^T) per 128-wide k-chunk, two
    heads packed into the PE array concurrently via row tiling (contraction
    is only d=64).
  - exp() runs on the scalar engine straight out of PSUM with the additive
    mask folded into the activation bias and 1/sqrt(d) into its scale.
  - The softmax denominator rides along the PV matmul as a 65th column of
    ones in the V operand.
  - The PV matmuls trail the score matmuls by 2 iterations in the PE stream
    (software pipelining), so the in-order PE never waits on the exp: the
    steady state is paced by the activation engine (~1.33us/iter).
  - Projection work is split into ~1024-cycle chunks drained evenly into the
    per-iteration PE slack (deadline-forced when attention needs them).
  - Normalization (1/denominator broadcast-multiply) runs on the otherwise
    idle GpSimd(Pool) engine + DVE, with only the 512-col broadcast matmul on
    the PE (via a spare slot of the scores-psum ring).
  - Matmul operands are fp16 (PE streams 2 bytes/cycle/partition, so fp32
    runs at half rate); accumulation stays fp32 in PSUM. Output is f16.
"""

import sys
import types

sys.path.insert(0, "/opt/trn_rl_repo")

import numpy as np

# NTFF profiling hook (missing from this image's antenv): only needed when
# tracing; install if available, degrade silently otherwise.
try:
    import antenv.axon_hooks  # noqa: F401
except ImportError:
    try:
        from trn_agent_boot.trn_boot import _ntff_profile_via_ctypes

        _m = types.ModuleType("antenv.axon_hooks")
        _hook = _ntff_profile_via_ctypes("/opt/axon/libaxon_pjrt.so")
        _m.get_axon_ntff_profile_hook = lambda: _hook
        _m.set_axon_ntff_profile_hook = lambda h: None
        sys.modules["antenv.axon_hooks"] = _m
    except Exception:
        pass

import concourse.tile as tile
from concourse import bacc, mybir
from concourse.tile_rust import add_dep_helper
from concourse.bass_utils import run_bass_kernel_spmd

F32 = mybir.dt.float32
F16 = mybir.dt.float16
EXP = mybir.ActivationFunctionType.Exp

B, S, H, NHEADS, D = 2, 2048, 1024, 16, 64
T = B * S                # 4096 tokens
DPC = 128                # output dims per core (2 heads x 64)
NCORES = 8
NKC = S // 128           # 16 k-chunks per batch
NQB = S // 512           # 4 q-blocks of 512 per batch
NTB = T // 512           # 8 token blocks of 512
NCI = H // 128           # 8 hidden (contraction) chunks
PVLAG = 12               # PV matmuls trail scores by this many iterations

last_exec_time_ns = None
last_results = None

_cache = {}


def _build():
    nc = bacc.Bacc(
        "TRN2", target_bir_lowering=False, debug=False, enable_asserts=False
    )
    # xt pre-blocked on host: [p, tb, ci, t] so each tb is contiguous/partition
    xt = nc.declare_dram_parameter("xt", [128, NTB, NCI, 512], F16,
                                   isOutput=False)
    wq = nc.declare_dram_parameter("wq", [128, NCI, 128], F16, isOutput=False)
    wk = nc.declare_dram_parameter("wk", [128, NCI, 128], F16, isOutput=False)
    wv = nc.declare_dram_parameter("wv", [128, NCI, 128], F16, isOutput=False)
    bq = nc.declare_dram_parameter("bq", [DPC, 1], F32, isOutput=False)
    bk = nc.declare_dram_parameter("bk", [DPC, 1], F32, isOutput=False)
    bvb = nc.declare_dram_parameter("bvb", [128, DPC], F32, isOutput=False)
    msk = nc.declare_dram_parameter("msk", [128, B * NKC], F32, isOutput=False)
    out = nc.declare_dram_parameter("out", [DPC, T], F16, isOutput=True)

    with tile.TileContext(nc) as tc:
        with tc.tile_pool(name="persist", bufs=1) as pp:
            wq_sb = pp.tile([128, NCI, 128], F16, tag="wq")
            wk_sb = pp.tile([128, NCI, 128], F16, tag="wk")
            wv_sb = pp.tile([128, NCI, 128], F16, tag="wv")
            bq_sb = pp.tile([DPC, 1], F32, tag="bq")
            bk_sb = pp.tile([DPC, 1], F32, tag="bk")
            bvb_sb = pp.tile([128, DPC], F32, tag="bvb")
            msk_sb = pp.tile([128, B * NKC], F32, tag="msk")
            ones_sb = pp.tile([128, 64], F16, tag="ones_sb")
            qt_sb = pp.tile([128, T], F16, tag="qt")
            kt_sb = pp.tile([128, T], F16, tag="kt")
            # V in [k, d] layout, a ones column per head: cols 0:64 = v0,
            # col 64 = ones, cols 65:129 = v1, col 129 = ones. PV head h
            # uses cols 65h:65h+65 ([v|1] -> ctx rows 0:64 + denom row 64).
            vx = pp.tile([128, B, NKC, 130], F16, tag="vx")
            # all 8 xt token-blocks resident; separate tiles for per-tb deps
            xts = [pp.tile([128, NCI, 512], F16, tag=f"xt{tb}",
                           name=f"xt{tb}") for tb in range(NTB)]

            # DMAs: K weights + first xt block first so K0 starts ASAP
            nc.scalar.dma_start(wk_sb[:], wk[:])
            nc.sync.dma_start(xts[0][:], xt[:, 0])
            nc.scalar.dma_start(wq_sb[:], wq[:])
            for tb in range(1, 4):
                nc.sync.dma_start(xts[tb][:], xt[:, tb])
            nc.scalar.dma_start(wv_sb[:], wv[:])
            nc.scalar.dma_start(bq_sb[:], bq[:])
            nc.scalar.dma_start(bk_sb[:], bk[:])
            nc.scalar.dma_start(bvb_sb[:], bvb[:])
            nc.scalar.dma_start(msk_sb[:], msk[:])
            for tb in range(4, NTB):
                nc.sync.dma_start(xts[tb][:], xt[:, tb])
            nc.vector.memset(ones_sb[:], 1.0)
            nc.vector.memset(
                vx[:].rearrange("p b k (h c) -> p b k h c", h=2,
                                c=65)[:, :, :, :, 64:65],
                1.0,
            )
            warm_sb = pp.tile([128, 512], F16, tag="warm_sb")
            nc.vector.memset(warm_sb[:], 0.5)

            with tc.tile_pool(name="stp", bufs=2, space="PSUM") as stp, \
                 tc.tile_pool(name="ctxp", bufs=2, space="PSUM") as ctxp, \
                 tc.tile_pool(name="fps", bufs=2, space="PSUM") as fps, \
                 tc.tile_pool(name="esp", bufs=16) as esp, \
                 tc.tile_pool(name="smallp", bufs=2) as smallp:
                qt_done = {}
                kt_done = {}
                vx_done = {}

                # ---- projection chunk emitters ----
                # Q/K: one psum [128,512] per token-block, filled by 4 chunks
                # of 2 ci-steps; bias-add on DVE completes it.
                def make_qk_chunks(kind, tb):
                    state = {}

                    def chunk(ci0, kind=kind, tb=tb, state=state):
                        w_sb, t_sb, b_sb = (
                            (wq_sb, qt_sb, bq_sb) if kind == "q"
                            else (wk_sb, kt_sb, bk_sb)
                        )
                        if ci0 == 0:
                            state["ps"] = fps.tile(
                                [128, 512], F32, tag="fps", name="pj_ps"
                            )
                        ps_t = state["ps"]
                        for ci in (ci0, ci0 + 1):
                            nc.tensor.matmul(
                                ps_t[:], w_sb[:, ci, :], xts[tb][:, ci, :],
                                start=(ci == 0), stop=(ci == NCI - 1),
                            )
                        if ci0 == NCI - 2:
                            col = tb * 512
                            done = (qt_done if kind == "q" else kt_done)
                            done[tb] = nc.vector.tensor_scalar_add(
                                t_sb[:, col:col + 512], ps_t[:], b_sb[:, 0:1]
                            )
                    return [(lambda c=ci0: chunk(c), 1024)
                            for ci0 in range(0, NCI, 2)]

                # V: per 128-token tt chunk, psum [128,128] filled by 2 halves
                # of 4 ci-steps; bias-adds on Pool complete vx.
                def make_v_chunks(tb):
                    state = {}

                    def vhalf(tt, half, tb=tb, state=state):
                        if half == 0:
                            state[tt] = fps.tile([128, 128], F32, tag="fps",
                                                 name="v_ps")
                        v_ps = state[tt]
                        for ci in range(half * 4, half * 4 + 4):
                            nc.tensor.matmul(
                                v_ps[:],
                                xts[tb][:, ci, tt * 128:(tt + 1) * 128],
                                wv_sb[:, ci, :],
                                start=(ci == 0), stop=(ci == NCI - 1),
                            )
                        if half == 1:
                            g = tb * 4 + tt
                            b_i, kc = g // NKC, g % NKC
                            # one op writes both heads around the shared
                            # ones column: dest cols {0:64} u {65:129}
                            dst = vx[:, b_i, kc, :].rearrange(
                                "p (h c) -> p h c", h=2, c=65
                            )[:, :, 0:64]
                            src = v_ps[:].rearrange(
                                "p (h c) -> p h c", h=2, c=64
                            )
                            bvr = bvb_sb[:].rearrange(
                                "p (h c) -> p h c", h=2, c=64
                            )
                            i0 = nc.vector.tensor_add(dst, src, bvr)
                            vx_done[(b_i, kc)] = (i0, i0)
                    out_chunks = []
                    for tt in range(4):
                        out_chunks.append((lambda t=tt: vhalf(t, 0), 512))
                        out_chunks.append((lambda t=tt: vhalf(t, 1), 512))
                    return out_chunks

                # ---- work queue: (deadline_iter, fn, cycles) ----
                # Groups are kept contiguous (atomic) so at most two
                # projection psum lifetimes ever interleave on the fps ring.
                work = []

                def add_qk(kind, tb, last_iter):
                    chunks = make_qk_chunks(kind, tb)
                    n = len(chunks)
                    for j, (fn, cyc) in enumerate(chunks):
                        work.append((last_iter - (n - 1 - j), fn, cyc))

                def add_v(tb):
                    # PV trails scores by PVLAG iterations, so vx(b,kc) is
                    # first consumed at iter b*64 + kc + PVLAG (margin 2)
                    b_i = tb // 4
                    for j, (fn, cyc) in enumerate(make_v_chunks(tb)):
                        tt = j // 2
                        work.append(
                            (b_i * 64 + 4 * (tb % 4) + tt + PVLAG - 2,
                             fn, cyc))

                # ordered by group-final deadline; K0/Q0 emitted upfront
                add_qk("k", 1, 2)
                add_qk("k", 2, 6)
                add_v(0)
                add_qk("k", 3, 10)
                add_v(1)
                add_qk("q", 1, 14)
                add_v(2)
                add_v(3)
                add_qk("q", 2, 30)
                add_qk("q", 3, 46)
                add_qk("k", 4, 55)
                add_qk("q", 4, 56)
                add_qk("k", 5, 62)
                add_v(4)
                add_qk("k", 6, 66)
                add_v(5)
                add_qk("k", 7, 70)
                add_v(6)
                add_qk("q", 5, 76)
                add_v(7)
                add_qk("q", 6, 92)
                add_qk("q", 7, 108)
                total_cycles = sum(c for _, _, c in work)
                # forcing must honor the earliest deadline anywhere in the
                # remaining (ordered) list: precompute suffix minima
                suffmin = [0] * len(work)
                m = 1 << 30
                for j in range(len(work) - 1, -1, -1):
                    m = min(m, work[j][0])
                    suffmin[j] = m

                pace_state = {"drained": 0, "pos": 0}

                def pace(i, niter):
                    target = total_cycles * (i + 1) // niter
                    while pace_state["pos"] < len(work) and (
                        suffmin[pace_state["pos"]] <= i
                        or pace_state["drained"] < target
                    ):
                        _, fn, cyc = work[pace_state["pos"]]
                        fn()
                        pace_state["pos"] += 1
                        pace_state["drained"] += cyc

                def drain_all():
                    while pace_state["pos"] < len(work):
                        _, fn, cyc = work[pace_state["pos"]]
                        fn()
                        pace_state["pos"] += 1

                # ---- PE p-state warmup ----
                # The PE clocks up only after ~3us of continuous execution;
                # run dummy matmuls while the first DMAs land so K0/Q0 and
                # the first score matmuls run at full clock.
                warm_ps = fps.tile([64, 512], F32, tag="fps",
                                   name="warm_ps")
                for _ in range(11):
                    nc.tensor.matmul(
                        warm_ps[:], warm_sb[:, 0:64], warm_sb[:],
                        start=True, stop=True,
                    )

                # ---- upfront projections: K0 then Q0 ----
                for fn, _ in make_qk_chunks("k", 0):
                    fn()
                for fn, _ in make_qk_chunks("q", 0):
                    fn()

                # ---- normalize (split into two stages) ----
                # stage A (Pool+DVE): copy ctx psum->sbuf f16; recip of denom
                # stage B (next iter): PE broadcast matmul + Pool multiply
                def norm_stage_a(blk):
                    b_i, qb, ctx0, ctx1 = blk
                    cs0 = smallp.tile([65, 512], F16, tag="cs0", name="cs0")
                    cs1 = smallp.tile([65, 512], F16, tag="cs1", name="cs1")
                    nc.vector.tensor_copy(cs0[:], ctx0[:])
                    nc.vector.tensor_copy(cs1[:], ctx1[:])
                    return (b_i, qb, cs0, cs1)

                def norm_head(h, cs, cout, qcol, dma_half=False):
                    bcp = fps.tile([64, 512], F32, tag="fps", name="bcp")
                    nc.tensor.matmul(
                        bcp[:], ones_sb[64:65, 0:64], cs[64:65, :],
                        start=True, stop=True, tile_position=(64, 0),
                    )
                    rb = smallp.tile([64, 512], F32, tag=f"rb{h}",
                                     name=f"rb{h}")
                    nc.vector.reciprocal_approx_fast(rb[:], bcp[:])
                    # all-SBUF multiply: run it on the idle Pool engine
                    nc.gpsimd.tensor_mul(
                        cout[h * 64:(h + 1) * 64, :], cs[0:64, :], rb[:]
                    )
                    if dma_half:
                        nc.sync.dma_start(
                            out[h * 64:(h + 1) * 64, qcol:qcol + 512],
                            cout[h * 64:(h + 1) * 64, :],
                        )

                def norm_stage_b(st_a):
                    b_i, qb, cs0, cs1 = st_a
                    qcol = b_i * S + qb * 512
                    cout = smallp.tile([128, 512], F16, tag="cout",
                                       name="cout")
                    norm_head(0, cs0, cout, qcol)
                    norm_head(1, cs1, cout, qcol)
                    nc.sync.dma_start(out[:, qcol:qcol + 512], cout[:])

                def norm_tail(blk):
                    # final block: shortest serial chain; act engine (idle
                    # by now) does the psum->sbuf copies instead of DVE
                    b_i, qb, ctx0, ctx1 = blk
                    qcol = b_i * S + qb * 512
                    cs0 = smallp.tile([65, 512], F16, tag="cs0", name="cs0")
                    cs1 = smallp.tile([65, 512], F16, tag="cs1", name="cs1")
                    cout = smallp.tile([128, 512], F16, tag="cout",
                                       name="cout")
                    nc.scalar.copy(cs0[:], ctx0[:])
                    nc.scalar.copy(cs1[:], ctx1[:])
                    norm_head(0, cs0, cout, qcol, dma_half=True)
                    norm_head(1, cs1, cout, qcol, dma_half=True)

                # ---- main loop ----
                iters = [(b_i, qb, kc) for b_i in range(B)
                         for qb in range(NQB) for kc in range(NKC)]
                niter = len(iters)
                pvq = []           # pending PV closures (lag PVLAG)
                stage_b_due = None
                stage_b_wait = 0   # iterations until stage B may be emitted

                def run_pv():
                    blk_done = pvq.pop(0)()
                    return blk_done

                for i, (b_i, qb, kc) in enumerate(iters):
                    if kc == 0:
                        ctx0 = ctxp.tile([65, 512], F32, tag="ctx",
                                         name="ctx0")
                        ctx1 = ctxp.tile([65, 512], F32, tag="ctx",
                                         name="ctx1")
                    qcol = b_i * S + qb * 512
                    kcol = b_i * S + kc * 128
                    ktb = b_i * 4 + kc // 4
                    qtb = b_i * 4 + qb
                    st = stp.tile([128, 1024], F32, tag="st", name="st")
                    m0 = nc.tensor.matmul(
                        st[:, 0:512],
                        kt_sb[0:64, kcol:kcol + 128],
                        qt_sb[0:64, qcol:qcol + 512],
                        start=True, stop=True, tile_position=(0, 0),
                    )
                    m1 = nc.tensor.matmul(
                        st[:, 512:1024],
                        kt_sb[64:128, kcol:kcol + 128],
                        qt_sb[64:128, qcol:qcol + 512],
                        start=True, stop=True, tile_position=(64, 0),
                    )
                    for m in (m0, m1):
                        add_dep_helper(m.ins, kt_done[ktb].ins,
                                       True, "kt ready")
                        add_dep_helper(m.ins, qt_done[qtb].ins,
                                       True, "qt ready")
                    est = esp.tile([128, 1024], F16, tag="est", name="est")
                    nc.scalar.activation(
                        est[:], st[:], EXP, scale=0.125,
                        bias=msk_sb[:, b_i * NKC + kc:b_i * NKC + kc + 1],
                    )

                    def make_pv(b_i=b_i, qb=qb, kc=kc, est=est,
                                ctx0=ctx0, ctx1=ctx1):
                        def pv():
                            p0 = nc.tensor.matmul(
                                ctx0[:], vx[:, b_i, kc, 0:65],
                                est[:, 0:512],
                                start=(kc == 0), stop=(kc == NKC - 1),
                            )
                            p1 = nc.tensor.matmul(
                                ctx1[:], vx[:, b_i, kc, 65:130],
                                est[:, 512:1024],
                                start=(kc == 0), stop=(kc == NKC - 1),
                            )
                            vd = vx_done[(b_i, kc)]
                            add_dep_helper(p0.ins, vd[0].ins, True, "vx0")
                            add_dep_helper(p1.ins, vd[1].ins, True, "vx1")
                            if kc == NKC - 1:
                                return (b_i, qb, ctx0, ctx1)
                            return None
                        return pv

                    # pop trailing PVs: one at steady lag, two per iteration
                    # in the final stretch so the tail stays short
                    blks = []
                    pops = 0
                    while pvq and (
                        len(pvq) >= PVLAG
                        or (i >= niter - 10 and pops < 2 and len(pvq) > 2)
                    ):
                        r = run_pv()
                        pops += 1
                        if r is not None:
                            blks.append(r)

                    pvq.append(make_pv())

                    if stage_b_due is not None:
                        if stage_b_wait > 0:
                            stage_b_wait -= 1
                        else:
                            norm_stage_b(stage_b_due)
                            stage_b_due = None
                    for blk in blks:
                        if stage_b_due is not None:
                            norm_stage_b(stage_b_due)
                        # give the DVE reciprocal chain ~3 iterations of
                        # headroom before the PE broadcast matmul needs it
                        stage_b_due = norm_stage_a(blk)
                        stage_b_wait = 3

                    # drain all filler by ~iter 112: b1 needs most of it
                    # well before the end, and the last stretch has no
                    # slack for bursts
                    pace(i, 96)

                # ---- tail ----
                drain_all()
                last_blk = None
                while pvq:
                    blk = run_pv()
                    if blk is not None:
                        last_blk = blk
                if stage_b_due is not None:
                    norm_stage_b(stage_b_due)
                if last_blk is not None:
                    norm_tail(last_blk)

    nc.compile()
    return nc


def kernel(hidden_states, attention_mask, Wq, bq, Wk, bk, Wv, bv, trace=False):
    global last_exec_time_ns, last_results
    x = np.asarray(hidden_states, dtype=np.float32)
    mask = np.asarray(attention_mask, dtype=np.float32)
    Wq = np.asarray(Wq, dtype=np.float32)
    Wk = np.asarray(Wk, dtype=np.float32)
    Wv = np.asarray(Wv, dtype=np.float32)
    bq = np.asarray(bq, dtype=np.float32)
    bk = np.asarray(bk, dtype=np.float32)
    bv = np.asarray(bv, dtype=np.float32)

    if "nc" not in _cache:
        _cache["nc"] = _build()
    nc = _cache["nc"]

    # xt host layout [p, tb, ci, t]: contiguous 8KB/partition per tb block
    xt4 = np.ascontiguousarray(
        x.reshape(T, H).T.astype(np.float16)          # [H, T]
        .reshape(NCI, 128, NTB, 512)                  # [ci, p, tb, t]
        .transpose(1, 2, 0, 3)                        # [p, tb, ci, t]
    )
    # mask columns: [p, b*16+kc] = mask[b, kc*128+p]
    mcols = np.ascontiguousarray(
        mask.reshape(B, NKC, 128).transpose(2, 0, 1).reshape(128, B * NKC)
    )

    def wpack(W, sl):
        # [H, 128] -> [p, ci, d]
        return np.ascontiguousarray(
            W[:, sl].astype(np.float16).reshape(NCI, 128, 128)
            .transpose(1, 0, 2)
        )

    in_maps = []
    for c in range(NCORES):
        sl = slice(c * DPC, (c + 1) * DPC)
        in_maps.append({
            "xt": xt4,
            "wq": wpack(Wq, sl),
            "wk": wpack(Wk, sl),
            "wv": wpack(Wv, sl),
            "bq": np.ascontiguousarray(bq[sl, None]),
            "bk": np.ascontiguousarray(bk[sl, None]),
            "bvb": np.ascontiguousarray(
                np.broadcast_to(bv[sl][None, :], (128, DPC))
            ),
            "msk": mcols,
        })

    res = run_bass_kernel_spmd(
        nc, in_maps, core_ids=list(range(NCORES)), trace=trace
    )
    last_exec_time_ns = res.exec_time_ns
    last_results = res

    # assemble: per-core out [128, T] f16 -> [B, S, 128]; concat over cores
    parts = [
        res.results[c]["out"].astype(np.float32)
        .reshape(DPC, B, S).transpose(1, 2, 0)
        for c in range(NCORES)
    ]
    return np.ascontiguousarray(np.concatenate(parts, axis=2))


# revision 39
# speedup vs baseline: 1.0280x; 1.0280x over previous
"""BERT self-attention (B=2, S=2048, H=1024, 16 heads) on 8 TRN2 NeuronCores.

Sharding: tensor-parallel over heads - 2 heads per core. Each core computes
Q/K/V projections for its head slice (contraction over the full hidden dim),
then attention for its (batch, head) pairs, producing the context transposed
[2*64, B*S]. The host concatenates the 8 per-core slices into [B, S, H].

Device-side layout choices:
  - X is fed pre-transposed and pre-blocked ([128, tb, ci, 512]) so each
    512-token block is one contiguous 8KB-per-partition DMA descriptor.
  - Scores are computed transposed (S^T = K Q^T) per 128-wide k-chunk, two
    heads packed into the PE array concurrently via row tiling (contraction
    is only d=64).
  - exp() runs on the scalar engine straight out of PSUM with the additive
    mask folded into the activation bias and 1/sqrt(d) into its scale.
  - The softmax denominator rides along the PV matmul as a 65th column of
    ones in the V operand.
  - The PV matmuls trail the score matmuls by 2 iterations in the PE stream
    (software pipelining), so the in-order PE never waits on the exp: the
    steady state is paced by the activation engine (~1.33us/iter).
  - Projection work is split into ~1024-cycle chunks drained evenly into the
    per-iteration PE slack (deadline-forced when attention needs them).
  - Normalization (1/denominator broadcast-multiply) runs on the otherwise
    idle GpSimd(Pool) engine + DVE, with only the 512-col broadcast matmul on
    the PE (via a spare slot of the scores-psum ring).
  - Matmul operands are fp16 (PE streams 2 bytes/cycle/partition, so fp32
    runs at half rate); accumulation stays fp32 in PSUM. Output is f16.
"""

import sys
import types

sys.path.insert(0, "/opt/trn_rl_repo")

import numpy as np

# NTFF profiling hook (missing from this image's antenv): only needed when
# tracing; install if available, degrade silently otherwise.
try:
    import antenv.axon_hooks  # noqa: F401
except ImportError:
    try:
        from trn_agent_boot.trn_boot import _ntff_profile_via_ctypes

        _m = types.ModuleType("antenv.axon_hooks")
        _hook = _ntff_profile_via_ctypes("/opt/axon/libaxon_pjrt.so")
        _m.get_axon_ntff_profile_hook = lambda: _hook
        _m.set_axon_ntff_profile_hook = lambda h: None
        sys.modules["antenv.axon_hooks"] = _m
    except Exception:
        pass

import concourse.tile as tile
from concourse import bacc, mybir
from concourse.tile_rust import add_dep_helper
from concourse.bass_utils import run_bass_kernel_spmd

F32 = mybir.dt.float32
F16 = mybir.dt.float16
EXP = mybir.ActivationFunctionType.Exp

B, S, H, NHEADS, D = 2, 2048, 1024, 16, 64
T = B * S                # 4096 tokens
DPC = 128                # output dims per core (2 heads x 64)
NCORES = 8
NKC = S // 128           # 16 k-chunks per batch
NQB = S // 512           # 4 q-blocks of 512 per batch
NTB = T // 512           # 8 token blocks of 512
NCI = H // 128           # 8 hidden (contraction) chunks
PVLAG = 12               # PV matmuls trail scores by this many iterations

last_exec_time_ns = None
last_results = None

_cache = {}


def _build():
    nc = bacc.Bacc(
        "TRN2", target_bir_lowering=False, debug=False, enable_asserts=False
    )
    # xt pre-blocked on host: [p, tb, ci, t] so each tb is contiguous/partition
    xt = nc.declare_dram_parameter("xt", [128, NTB, NCI, 512], F16,
                                   isOutput=False)
    wq = nc.declare_dram_parameter("wq", [128, NCI, 128], F16, isOutput=False)
    wk = nc.declare_dram_parameter("wk", [128, NCI, 128], F16, isOutput=False)
    wv = nc.declare_dram_parameter("wv", [128, NCI, 128], F16, isOutput=False)
    bq = nc.declare_dram_parameter("bq", [DPC, 1], F32, isOutput=False)
    bk = nc.declare_dram_parameter("bk", [DPC, 1], F32, isOutput=False)
    bvb = nc.declare_dram_parameter("bvb", [128, DPC], F32, isOutput=False)
    msk = nc.declare_dram_parameter("msk", [128, B * NKC], F32, isOutput=False)
    out = nc.declare_dram_parameter("out", [DPC, T], F16, isOutput=True)

    with tile.TileContext(nc) as tc:
        with tc.tile_pool(name="persist", bufs=1) as pp:
            wq_sb = pp.tile([128, NCI, 128], F16, tag="wq")
            wk_sb = pp.tile([128, NCI, 128], F16, tag="wk")
            wv_sb = pp.tile([128, NCI, 128], F16, tag="wv")
            bq_sb = pp.tile([DPC, 1], F32, tag="bq")
            bk_sb = pp.tile([DPC, 1], F32, tag="bk")
            bvb_sb = pp.tile([128, DPC], F32, tag="bvb")
            msk_sb = pp.tile([128, B * NKC], F32, tag="msk")
            ones_sb = pp.tile([128, 64], F16, tag="ones_sb")
            qt_sb = pp.tile([128, T], F16, tag="qt")
            kt_sb = pp.tile([128, T], F16, tag="kt")
            # V in [k, d] layout, a ones column per head: cols 0:64 = v0,
            # col 64 = ones, cols 65:129 = v1, col 129 = ones. PV head h
            # uses cols 65h:65h+65 ([v|1] -> ctx rows 0:64 + denom row 64).
            vx = pp.tile([128, B, NKC, 130], F16, tag="vx")
            # all 8 xt token-blocks resident; separate tiles for per-tb deps
            xts = [pp.tile([128, NCI, 512], F16, tag=f"xt{tb}",
                           name=f"xt{tb}") for tb in range(NTB)]

            # DMAs: K weights + first xt block first so K0 starts ASAP; the
            # tiny bias/mask tensors go next (the first bias-add gates the
            # first score matmul), bulk xt after
            nc.scalar.dma_start(wk_sb[:], wk[:])
            nc.sync.dma_start(xts[0][:], xt[:, 0])
            nc.scalar.dma_start(wq_sb[:], wq[:])
            nc.scalar.dma_start(bk_sb[:], bk[:])
            nc.scalar.dma_start(bq_sb[:], bq[:])
            nc.scalar.dma_start(bvb_sb[:], bvb[:])
            nc.scalar.dma_start(msk_sb[:], msk[:])
            for tb in range(1, 4):
                nc.sync.dma_start(xts[tb][:], xt[:, tb])
            nc.scalar.dma_start(wv_sb[:], wv[:])
            for tb in range(4, NTB):
                nc.sync.dma_start(xts[tb][:], xt[:, tb])
            nc.vector.memset(ones_sb[:], 1.0)
            nc.vector.memset(
                vx[:].rearrange("p b k (h c) -> p b k h c", h=2,
                                c=65)[:, :, :, :, 64:65],
                1.0,
            )
            warm_sb = pp.tile([128, 512], F16, tag="warm_sb")
            nc.vector.memset(warm_sb[:], 0.5)

            with tc.tile_pool(name="stp", bufs=2, space="PSUM") as stp, \
                 tc.tile_pool(name="ctxp", bufs=2, space="PSUM") as ctxp, \
                 tc.tile_pool(name="fps", bufs=2, space="PSUM") as fps, \
                 tc.tile_pool(name="esp", bufs=16) as esp, \
                 tc.tile_pool(name="smallp", bufs=2) as smallp:
                qt_done = {}
                kt_done = {}
                vx_done = {}

                # ---- projection chunk emitters ----
                # Q/K: one psum [128,512] per token-block, filled by 4 chunks
                # of 2 ci-steps; bias-add on DVE completes it.
                def make_qk_chunks(kind, tb):
                    state = {}

                    def chunk(ci0, kind=kind, tb=tb, state=state):
                        w_sb, t_sb, b_sb = (
                            (wq_sb, qt_sb, bq_sb) if kind == "q"
                            else (wk_sb, kt_sb, bk_sb)
                        )
                        if ci0 == 0:
                            state["ps"] = fps.tile(
                                [128, 512], F32, tag="fps", name="pj_ps"
                            )
                        ps_t = state["ps"]
                        for ci in (ci0, ci0 + 1):
                            nc.tensor.matmul(
                                ps_t[:], w_sb[:, ci, :], xts[tb][:, ci, :],
                                start=(ci == 0), stop=(ci == NCI - 1),
                            )
                        if ci0 == NCI - 2:
                            col = tb * 512
                            done = (qt_done if kind == "q" else kt_done)
                            done[tb] = nc.vector.tensor_scalar_add(
                                t_sb[:, col:col + 512], ps_t[:], b_sb[:, 0:1]
                            )
                    return [(lambda c=ci0: chunk(c), 1024)
                            for ci0 in range(0, NCI, 2)]

                # V: per 128-token tt chunk, psum [128,128] filled by 2 halves
                # of 4 ci-steps; bias-adds on Pool complete vx.
                def make_v_chunks(tb):
                    state = {}

                    def vhalf(tt, half, tb=tb, state=state):
                        if half == 0:
                            state[tt] = fps.tile([128, 128], F32, tag="fps",
                                                 name="v_ps")
                        v_ps = state[tt]
                        for ci in range(half * 4, half * 4 + 4):
                            nc.tensor.matmul(
                                v_ps[:],
                                xts[tb][:, ci, tt * 128:(tt + 1) * 128],
                                wv_sb[:, ci, :],
                                start=(ci == 0), stop=(ci == NCI - 1),
                            )
                        if half == 1:
                            g = tb * 4 + tt
                            b_i, kc = g // NKC, g % NKC
                            # one op writes both heads around the shared
                            # ones column: dest cols {0:64} u {65:129}
                            dst = vx[:, b_i, kc, :].rearrange(
                                "p (h c) -> p h c", h=2, c=65
                            )[:, :, 0:64]
                            src = v_ps[:].rearrange(
                                "p (h c) -> p h c", h=2, c=64
                            )
                            bvr = bvb_sb[:].rearrange(
                                "p (h c) -> p h c", h=2, c=64
                            )
                            i0 = nc.vector.tensor_add(dst, src, bvr)
                            vx_done[(b_i, kc)] = (i0, i0)
                    out_chunks = []
                    for tt in range(4):
                        out_chunks.append((lambda t=tt: vhalf(t, 0), 512))
                        out_chunks.append((lambda t=tt: vhalf(t, 1), 512))
                    return out_chunks

                # ---- work queue: (deadline_iter, fn, cycles) ----
                # Groups are kept contiguous (atomic) so at most two
                # projection psum lifetimes ever interleave on the fps ring.
                work = []

                def add_qk(kind, tb, last_iter):
                    chunks = make_qk_chunks(kind, tb)
                    n = len(chunks)
                    for j, (fn, cyc) in enumerate(chunks):
                        work.append((last_iter - (n - 1 - j), fn, cyc))

                def add_v(tb):
                    # PV trails scores by PVLAG iterations, so vx(b,kc) is
                    # first consumed at iter b*64 + kc + PVLAG (margin 2)
                    b_i = tb // 4
                    for j, (fn, cyc) in enumerate(make_v_chunks(tb)):
                        tt = j // 2
                        work.append(
                            (b_i * 64 + 4 * (tb % 4) + tt + PVLAG - 2,
                             fn, cyc))

                # ordered by group-final deadline; K0/Q0 emitted upfront
                add_qk("k", 1, 2)
                add_qk("k", 2, 6)
                add_v(0)
                add_qk("k", 3, 10)
                add_v(1)
                add_qk("q", 1, 14)
                add_v(2)
                add_v(3)
                add_qk("q", 2, 30)
                add_qk("q", 3, 46)
                add_qk("k", 4, 55)
                add_qk("q", 4, 56)
                add_qk("k", 5, 62)
                add_v(4)
                add_qk("k", 6, 66)
                add_v(5)
                add_qk("k", 7, 70)
                add_v(6)
                add_qk("q", 5, 76)
                add_v(7)
                add_qk("q", 6, 92)
                add_qk("q", 7, 108)
                total_cycles = sum(c for _, _, c in work)
                # forcing must honor the earliest deadline anywhere in the
                # remaining (ordered) list: precompute suffix minima
                suffmin = [0] * len(work)
                m = 1 << 30
                for j in range(len(work) - 1, -1, -1):
                    m = min(m, work[j][0])
                    suffmin[j] = m

                pace_state = {"drained": 0, "pos": 0}

                def pace(i, niter):
                    target = total_cycles * (i + 1) // niter
                    while pace_state["pos"] < len(work) and (
                        suffmin[pace_state["pos"]] <= i
                        or pace_state["drained"] < target
                    ):
                        _, fn, cyc = work[pace_state["pos"]]
                        fn()
                        pace_state["pos"] += 1
                        pace_state["drained"] += cyc

                def drain_all():
                    while pace_state["pos"] < len(work):
                        _, fn, cyc = work[pace_state["pos"]]
                        fn()
                        pace_state["pos"] += 1

                # ---- PE p-state warmup ----
                # The PE clocks up only after ~3us of continuous execution;
                # run dummy matmuls while the first DMAs land so K0/Q0 and
                # the first score matmuls run at full clock.
                warm_ps = fps.tile([64, 512], F32, tag="fps",
                                   name="warm_ps")
                for _ in range(13):
                    nc.tensor.matmul(
                        warm_ps[:], warm_sb[:, 0:64], warm_sb[:],
                        start=True, stop=True,
                    )

                # ---- upfront projections: K0 then Q0 ----
                for fn, _ in make_qk_chunks("k", 0):
                    fn()
                for fn, _ in make_qk_chunks("q", 0):
                    fn()

                # ---- normalize (split into two stages) ----
                # stage A (Pool+DVE): copy ctx psum->sbuf f16; recip of denom
                # stage B (next iter): PE broadcast matmul + Pool multiply
                def norm_stage_a(blk):
                    b_i, qb, ctx0, ctx1 = blk
                    cs0 = smallp.tile([65, 512], F16, tag="cs0", name="cs0")
                    cs1 = smallp.tile([65, 512], F16, tag="cs1", name="cs1")
                    nc.vector.tensor_copy(cs0[:], ctx0[:])
                    nc.vector.tensor_copy(cs1[:], ctx1[:])
                    return (b_i, qb, cs0, cs1)

                def norm_head(h, cs, cout, qcol, dma_half=False):
                    bcp = fps.tile([64, 512], F32, tag="fps", name="bcp")
                    nc.tensor.matmul(
                        bcp[:], ones_sb[64:65, 0:64], cs[64:65, :],
                        start=True, stop=True, tile_position=(64, 0),
                    )
                    rb = smallp.tile([64, 512], F32, tag=f"rb{h}",
                                     name=f"rb{h}")
                    nc.vector.reciprocal_approx_fast(rb[:], bcp[:])
                    # all-SBUF multiply: run it on the idle Pool engine
                    nc.gpsimd.tensor_mul(
                        cout[h * 64:(h + 1) * 64, :], cs[0:64, :], rb[:]
                    )
                    if dma_half:
                        nc.sync.dma_start(
                            out[h * 64:(h + 1) * 64, qcol:qcol + 512],
                            cout[h * 64:(h + 1) * 64, :],
                        )

                def norm_stage_b(st_a):
                    b_i, qb, cs0, cs1 = st_a
                    qcol = b_i * S + qb * 512
                    cout = smallp.tile([128, 512], F16, tag="cout",
                                       name="cout")
                    norm_head(0, cs0, cout, qcol)
                    norm_head(1, cs1, cout, qcol)
                    nc.sync.dma_start(out[:, qcol:qcol + 512], cout[:])

                def norm_tail(blk):
                    # final block: shortest serial chain; act engine (idle
                    # by now) does the psum->sbuf copies instead of DVE
                    b_i, qb, ctx0, ctx1 = blk
                    qcol = b_i * S + qb * 512
                    cs0 = smallp.tile([65, 512], F16, tag="cs0", name="cs0")
                    cs1 = smallp.tile([65, 512], F16, tag="cs1", name="cs1")
                    cout = smallp.tile([128, 512], F16, tag="cout",
                                       name="cout")
                    nc.scalar.copy(cs0[:], ctx0[:])
                    nc.scalar.copy(cs1[:], ctx1[:])
                    norm_head(0, cs0, cout, qcol, dma_half=True)
                    norm_head(1, cs1, cout, qcol, dma_half=True)

                # ---- main loop ----
                iters = [(b_i, qb, kc) for b_i in range(B)
                         for qb in range(NQB) for kc in range(NKC)]
                niter = len(iters)
                pvq = []           # pending PV closures (lag PVLAG)
                stage_b_due = None
                stage_b_wait = 0   # iterations until stage B may be emitted

                def run_pv():
                    blk_done = pvq.pop(0)()
                    return blk_done

                for i, (b_i, qb, kc) in enumerate(iters):
                    if kc == 0:
                        ctx0 = ctxp.tile([65, 512], F32, tag="ctx",
                                         name="ctx0")
                        ctx1 = ctxp.tile([65, 512], F32, tag="ctx",
                                         name="ctx1")
                    qcol = b_i * S + qb * 512
                    kcol = b_i * S + kc * 128
                    ktb = b_i * 4 + kc // 4
                    qtb = b_i * 4 + qb
                    st = stp.tile([128, 1024], F32, tag="st", name="st")
                    m0 = nc.tensor.matmul(
                        st[:, 0:512],
                        kt_sb[0:64, kcol:kcol + 128],
                        qt_sb[0:64, qcol:qcol + 512],
                        start=True, stop=True, tile_position=(0, 0),
                    )
                    m1 = nc.tensor.matmul(
                        st[:, 512:1024],
                        kt_sb[64:128, kcol:kcol + 128],
                        qt_sb[64:128, qcol:qcol + 512],
                        start=True, stop=True, tile_position=(64, 0),
                    )
                    for m in (m0, m1):
                        add_dep_helper(m.ins, kt_done[ktb].ins,
                                       True, "kt ready")
                        add_dep_helper(m.ins, qt_done[qtb].ins,
                                       True, "qt ready")
                    est = esp.tile([128, 1024], F16, tag="est", name="est")
                    nc.scalar.activation(
                        est[:], st[:], EXP, scale=0.125,
                        bias=msk_sb[:, b_i * NKC + kc:b_i * NKC + kc + 1],
                    )

                    def make_pv(b_i=b_i, qb=qb, kc=kc, est=est,
                                ctx0=ctx0, ctx1=ctx1):
                        def pv():
                            p0 = nc.tensor.matmul(
                                ctx0[:], vx[:, b_i, kc, 0:65],
                                est[:, 0:512],
                                start=(kc == 0), stop=(kc == NKC - 1),
                            )
                            p1 = nc.tensor.matmul(
                                ctx1[:], vx[:, b_i, kc, 65:130],
                                est[:, 512:1024],
                                start=(kc == 0), stop=(kc == NKC - 1),
                            )
                            vd = vx_done[(b_i, kc)]
                            add_dep_helper(p0.ins, vd[0].ins, True, "vx0")
                            add_dep_helper(p1.ins, vd[1].ins, True, "vx1")
                            if kc == NKC - 1:
                                return (b_i, qb, ctx0, ctx1)
                            return None
                        return pv

                    # pop trailing PVs: one at steady lag, two per iteration
                    # in the final stretch so the tail stays short
                    blks = []
                    pops = 0
                    while pvq and (
                        len(pvq) >= PVLAG
                        or (i >= niter - 10 and pops < 2 and len(pvq) > 2)
                    ):
                        r = run_pv()
                        pops += 1
                        if r is not None:
                            blks.append(r)

                    pvq.append(make_pv())

                    if stage_b_due is not None:
                        if stage_b_wait > 0:
                            stage_b_wait -= 1
                        else:
                            norm_stage_b(stage_b_due)
                            stage_b_due = None
                    for blk in blks:
                        if stage_b_due is not None:
                            norm_stage_b(stage_b_due)
                        # give the DVE reciprocal chain ~3 iterations of
                        # headroom before the PE broadcast matmul needs it
                        stage_b_due = norm_stage_a(blk)
                        stage_b_wait = 3

                    # drain all filler by ~iter 112: b1 needs most of it
                    # well before the end, and the last stretch has no
                    # slack for bursts
                    pace(i, 96)

                # ---- tail ----
                drain_all()
                last_blk = None
                while pvq:
                    blk = run_pv()
                    if blk is not None:
                        last_blk = blk
                if stage_b_due is not None:
                    norm_stage_b(stage_b_due)
                if last_blk is not None:
                    norm_tail(last_blk)

    nc.compile()
    return nc


def kernel(hidden_states, attention_mask, Wq, bq, Wk, bk, Wv, bv, trace=False):
    global last_exec_time_ns, last_results
    x = np.asarray(hidden_states, dtype=np.float32)
    mask = np.asarray(attention_mask, dtype=np.float32)
    Wq = np.asarray(Wq, dtype=np.float32)
    Wk = np.asarray(Wk, dtype=np.float32)
    Wv = np.asarray(Wv, dtype=np.float32)
    bq = np.asarray(bq, dtype=np.float32)
    bk = np.asarray(bk, dtype=np.float32)
    bv = np.asarray(bv, dtype=np.float32)

    if "nc" not in _cache:
        _cache["nc"] = _build()
    nc = _cache["nc"]

    # xt host layout [p, tb, ci, t]: contiguous 8KB/partition per tb block
    xt4 = np.ascontiguousarray(
        x.reshape(T, H).T.astype(np.float16)          # [H, T]
        .reshape(NCI, 128, NTB, 512)                  # [ci, p, tb, t]
        .transpose(1, 2, 0, 3)                        # [p, tb, ci, t]
    )
    # mask columns: [p, b*16+kc] = mask[b, kc*128+p]
    mcols = np.ascontiguousarray(
        mask.reshape(B, NKC, 128).transpose(2, 0, 1).reshape(128, B * NKC)
    )

    def wpack(W, sl):
        # [H, 128] -> [p, ci, d]
        return np.ascontiguousarray(
            W[:, sl].astype(np.float16).reshape(NCI, 128, 128)
            .transpose(1, 0, 2)
        )

    in_maps = []
    for c in range(NCORES):
        sl = slice(c * DPC, (c + 1) * DPC)
        in_maps.append({
            "xt": xt4,
            "wq": wpack(Wq, sl),
            "wk": wpack(Wk, sl),
            "wv": wpack(Wv, sl),
            "bq": np.ascontiguousarray(bq[sl, None]),
            "bk": np.ascontiguousarray(bk[sl, None]),
            "bvb": np.ascontiguousarray(
                np.broadcast_to(bv[sl][None, :], (128, DPC))
            ),
            "msk": mcols,
        })

    res = run_bass_kernel_spmd(
        nc, in_maps, core_ids=list(range(NCORES)), trace=trace
    )
    last_exec_time_ns = res.exec_time_ns
    last_results = res

    # assemble: per-core out [128, T] f16 -> [B, S, 128]; concat over cores
    parts = [
        res.results[c]["out"].astype(np.float32)
        .reshape(DPC, B, S).transpose(1, 2, 0)
        for c in range(NCORES)
    ]
    return np.ascontiguousarray(np.concatenate(parts, axis=2))
